# revision 1
# baseline (speedup 1.0000x reference)
"""Multi-head attention (B=4, S=2048, D=1024, H=16, hd=64) with RoPE on 8 trn2 cores.

Sharding: core c handles batch b=c//2, head-group hg=c%2 (8 heads, 512 features).
Each core computes y_partial.T = Wo[:, fslice] @ ctx.T for its heads; the host
sums the two partials per batch and adds bo.

Device layout (all "transposed" so the contraction dim sits on partitions):
  xT   [D=1024, S=2048]   hidden.T (host-pretransposed)
  wqT/wkT/wvT [1024, 512] W[fslice,:].T
  Q.T/K.T [512, 2048]     per-core projections, RoPE applied in place
  V'   [S=2048, 8, 65]    natural-layout V per head + ones column (softmax denom)
  scores.T [k, q] = K.T^T @ Q.T chunks, exp on ACT w/ scale 1/8, bias -8
  PV: ctx.T[hd+1, q] = V'^T @ expS.T  (row 64 = denominator)
  out: y.T[e, q] = Wo.T^T @ (ctx.T * 1/den)
"""

import contextlib

import numpy as np

import concourse.bass as bass
import concourse.mybir as mybir
import concourse.tile as tile
from concourse import bacc
from concourse.bass_utils import run_bass_kernel_spmd

F32 = mybir.dt.float32
F32R = mybir.dt.float32r
AF = mybir.ActivationFunctionType
ADD = mybir.AluOpType.add
MULT = mybir.AluOpType.mult

B, S, D, H = 4, 2048, 1024, 16
HD = D // H            # 64
NCORES = 8
FC = D // 2            # 512 features (8 heads) per core (2 cores per batch)
NH = FC // HD          # 8 heads per core
QN = 256               # q-chunk width for attention/out-proj matmuls
SN = 512               # s-chunk width for projection matmuls
NQC = S // QN          # 8
NKC = S // 128         # 16 k-chunks
NDC = D // 128         # 8 d-chunks
NFC = FC // 128        # 4 f-chunks (head pairs)
EXP_BIAS = -8.0        # constant shift inside exp (cancels in softmax)
SCALE = 1.0 / np.sqrt(HD)


def build_kernel(dump=False, repeat=1):
    nc = bacc.Bacc("TRN2", debug=False)

    XS = 256  # x slab width
    xp = nc.dram_tensor("xp", [S // XS, 128, NDC, XS], F32R, kind="ExternalInput")
    wq = nc.dram_tensor("wq", [128, NDC, FC], F32R, kind="ExternalInput")
    wk = nc.dram_tensor("wk", [128, NDC, FC], F32R, kind="ExternalInput")
    wv = nc.dram_tensor("wv", [128, NDC, FC], F32R, kind="ExternalInput")
    wo = nc.dram_tensor("wo", [128, NFC, D], F32R, kind="ExternalInput")
    bq = nc.dram_tensor("bq", [FC], F32, kind="ExternalInput")
    bk = nc.dram_tensor("bk", [FC], F32, kind="ExternalInput")
    bv = nc.dram_tensor("bv", [1, FC], F32R, kind="ExternalInput")
    c2 = nc.dram_tensor("c2", [128, S], F32, kind="ExternalInput")
    s2 = nc.dram_tensor("s2", [128, S], F32, kind="ExternalInput")
    onesin = nc.dram_tensor("onesin", [1, S], F32R, kind="ExternalInput")
    perm = nc.dram_tensor("perm", [128, 128], F32R, kind="ExternalInput")
    yT = nc.dram_tensor("yT", [D, S], F32, kind="ExternalOutput")
    if dump:
        qT_d = nc.dram_tensor("qT_d", [FC, S], F32, kind="ExternalOutput")
        kT_d = nc.dram_tensor("kT_d", [FC, S], F32, kind="ExternalOutput")
        vt_d = nc.dram_tensor("vt_d", [S, NH, HD + 1], F32, kind="ExternalOutput")

    NG = 4  # kc-chunks per exp slab

    with tile.TileContext(nc) as tc:
      for _rep in range(repeat):
       with contextlib.ExitStack() as ctx:
        ll = ctx.enter_context(tc.tile_pool(name="ll", bufs=1))

        qT = [ll.tile([128, S], F32R, name=f"qT{i}") for i in range(NFC)]
        kT = [ll.tile([128, S], F32R, name=f"kT{i}") for i in range(NFC)]
        ebias = ll.tile([128, 1], F32, name="ebias")
        ones_sb = ll.tile([1, S], F32R, name="ones_sb")
        ones_col = ll.tile([128, NH], F32R, name="ones_col")
        perm_sb = ll.tile([128, 128], F32R, name="perm_sb")
        bqs = ll.tile([128, NFC], F32, name="bqs")
        bks = ll.tile([128, NFC], F32, name="bks")

        nc.vector.memset(ebias, EXP_BIAS)
        nc.sync.dma_start(out=ones_sb, in_=onesin[:])
        ones_dram = onesin[:]
        nc.sync.dma_start(
            out=ones_col,
            in_=bass.AP(tensor=ones_dram.tensor, offset=ones_dram.offset,
                        ap=[[0, 128], [1, NH]]))
        nc.sync.dma_start(out=perm_sb, in_=perm[:])
        nc.sync.dma_start(out=bqs, in_=bq[:].rearrange("(c p) -> p c", p=128))
        nc.sync.dma_start(out=bks, in_=bk[:].rearrange("(c p) -> p c", p=128))

        # ---------------- Phase A: Q/K/V projections + RoPE (one xp pass) ----------------
        ll2 = ctx.enter_context(tc.tile_pool(name="ll2", bufs=1))
        vt = [ll2.tile([128, NH, HD + 1], F32R, name=f"vt{k}") for k in range(NKC)]
        xp_r = xp[:]
        with tc.tile_pool(name="pA", bufs=1) as pA, \
             tc.tile_pool(name="ppA", bufs=2, space="PSUM") as ppA:
            c2_sb = pA.tile([128, S], F32, name="c2_sb")
            s2_sb = pA.tile([128, S], F32, name="s2_sb")
            nc.sync.dma_start(out=c2_sb, in_=c2[:])
            nc.sync.dma_start(out=s2_sb, in_=s2[:])
            with tc.tile_pool(name="wqk", bufs=1) as wqk:
                wq_sb = wqk.tile([128, NDC, FC], F32R, name="wq_sb")
                wk_sb = wqk.tile([128, NDC, FC], F32R, name="wk_sb")
                wv_sb = wqk.tile([128, NDC, FC], F32R, name="wv_sb")
                bv_sb = wqk.tile([1, FC], F32R, name="bv_sb")
                nc.sync.dma_start(out=bv_sb, in_=bv[:])
                nc.sync.dma_start(out=wq_sb, in_=wq[:])
                nc.sync.dma_start(out=wk_sb, in_=wk[:])
                nc.sync.dma_start(out=wv_sb, in_=wv[:])

                for scg in range(S // XS):
                    xh = pA.tile([128, NDC, XS], F32R, name="xh", tag="xh", bufs=2)
                    nc.sync.dma_start(out=xh, in_=xp_r[scg])
                    sg = scg * XS
                    for fc in range(NFC):
                        for w_sb, bias_t, out_t in (
                            (wq_sb, bqs, qT),
                            (wk_sb, bks, kT),
                        ):
                            ps = ppA.tile([128, XS], F32, name="ps", tag="proj",
                                          bufs=3)
                            for d in range(NDC):
                                nc.tensor.matmul(
                                    ps,
                                    w_sb[:, d, fc * 128:(fc + 1) * 128],
                                    xh[:, d, :],
                                    start=(d == 0),
                                    stop=(d == NDC - 1),
                                )
                            t = out_t[fc]
                            praw = pA.tile(
                                [128, XS], F32R, name="praw", tag="praw", bufs=3,
                            )
                            nc.vector.tensor_scalar(
                                praw, ps, bias_t[:, fc:fc + 1], None, op0=ADD,
                            )
                            # RoPE: t_slice = praw*C2 + (P.T @ praw)*S2
                            swps = ppA.tile(
                                [128, XS], F32, name="swps", tag="swp", bufs=2,
                            )
                            nc.tensor.matmul(
                                swps, perm_sb, praw, start=True, stop=True,
                            )
                            prod = pA.tile(
                                [128, XS], F32, name="prod", tag="prod", bufs=3,
                            )
                            nc.vector.tensor_mul(
                                prod, swps, s2_sb[:, sg:sg + XS])
                            nc.vector.tensor_mul(
                                t[:, sg:sg + XS], praw, c2_sb[:, sg:sg + XS])
                            nc.gpsimd.tensor_add(
                                t[:, sg:sg + XS], t[:, sg:sg + XS], prod)
                    for ss in range(XS // 128):
                        kg = scg * (XS // 128) + ss
                        psv = ppA.tile([128, FC], F32, name="psv", tag="projv")
                        for d in range(NDC):
                            nc.tensor.matmul(
                                psv,
                                xh[:, d, ss * 128:(ss + 1) * 128],
                                wv_sb[:, d, :],
                                start=(d == 0),
                                stop=False,
                            )
                        nc.tensor.matmul(
                            psv, ones_sb[0:1, 0:128], bv_sb, start=False, stop=True,
                        )
                        nc.vector.tensor_copy(
                            vt[kg][:, :, 0:HD],
                            psv.rearrange("p (h e) -> p h e", e=HD),
                        )
                        nc.vector.tensor_copy(
                            vt[kg][:, :, HD:HD + 1],
                            ones_col.rearrange("p (h o) -> p h o", o=1),
                        )
            if dump:
                qd_r = qT_d[:].rearrange("(c p) s -> c p s", p=128)
                kd_r = kT_d[:].rearrange("(c p) s -> c p s", p=128)
                for fc in range(NFC):
                    nc.sync.dma_start(out=qd_r[fc], in_=qT[fc].bitcast(F32))
                    nc.sync.dma_start(out=kd_r[fc], in_=kT[fc].bitcast(F32))
                for kg in range(NKC):
                    nc.sync.dma_start(
                        out=vt_d[:].rearrange("(c p) h e -> c p h e", p=128)[kg],
                        in_=vt[kg].bitcast(F32),
                    )

        # ---------------- Phase 3: attention + output projection ----------------
        with tc.tile_pool(name="p3", bufs=1) as p3, \
             tc.tile_pool(name="ps_sc", bufs=2, space="PSUM") as ps_sc, \
             tc.tile_pool(name="ps_pv", bufs=2, space="PSUM") as ps_pv, \
             tc.tile_pool(name="ps_o", bufs=2, space="PSUM") as ps_o:
            wo_sb = p3.tile([128, NFC, D], F32R, name="wo_sb")
            nc.sync.dma_start(out=wo_sb, in_=wo[:])
            for qc in range(NQC):
                q_sl = slice(qc * QN, (qc + 1) * QN)
                ctxT = []
                for pair in range(NFC):
                    exp_sl = [[None] * NG for _ in range(2)]
                    for g in range(NG):
                        sc_ps = [
                            ps_sc.tile([128, NG, QN], F32, name="sc_ps", tag="sc")
                            for h in range(2)
                        ]
                        for j in range(NG):
                            kc = g * NG + j
                            k_sl = slice(kc * 128, (kc + 1) * 128)
                            for h in range(2):
                                nc.tensor.matmul(
                                    sc_ps[h][:, j, :],
                                    kT[pair][h * 64:(h + 1) * 64, k_sl],
                                    qT[pair][h * 64:(h + 1) * 64, q_sl],
                                    start=True, stop=True,
                                    tile_position=(h * 64, 0),
                                )
                        for h in range(2):
                            es = p3.tile(
                                [128, NG, QN], F32R, name="es", tag=f"exp{h}{g}",
                                bufs=2,
                            )
                            nc.scalar.activation(
                                es, sc_ps[h], AF.Exp, bias=ebias, scale=SCALE,
                            )
                            exp_sl[h][g] = es
                    for h in range(2):
                        hh = pair * 2 + h
                        pv_ps = ps_pv.tile([128, QN], F32, name="pv_ps", tag="pv")
                        for kc in range(NKC):
                            nc.tensor.matmul(
                                pv_ps[0:HD + 1, :],
                                vt[kc][:, hh, :],
                                exp_sl[h][kc // NG][:, kc % NG, :],
                                start=(kc == 0),
                                stop=(kc == NKC - 1),
                            )
                        denr = p3.tile([1, QN], F32, name="denr", tag="denr")
                        nc.vector.reciprocal(denr, pv_ps[HD:HD + 1, :])
                        denb = p3.tile([64, QN], F32, name="denb", tag="denb")
                        nc.gpsimd.partition_broadcast(denb, denr)
                        if h == 0:
                            ct = p3.tile([128, QN], F32R, name="ct", tag=f"ctx{pair}")
                            ctxT.append(ct)
                        nc.vector.tensor_tensor(
                            ctxT[pair][h * 64:(h + 1) * 64, :],
                            pv_ps[0:HD, :], denb, op=MULT,
                        )
                for ec in range(NDC):
                    ops = ps_o.tile([128, QN], F32, name="ops", tag="out")
                    for fc in range(NFC):
                        nc.tensor.matmul(
                            ops,
                            wo_sb[:, fc, ec * 128:(ec + 1) * 128],
                            ctxT[fc],
                            start=(fc == 0),
                            stop=(fc == NFC - 1),
                        )
                    ysb = p3.tile([128, QN], F32, name="ysb", tag="y", bufs=3)
                    nc.vector.tensor_copy(ysb, ops)
                    nc.sync.dma_start(
                        out=yT[:].rearrange("(c p) s -> c p s", p=128)[ec, :, q_sl],
                        in_=ysb,
                    )

    nc.finalize()
    return nc


def _rope_tables():
    inv_freq = 1.0 / (10000.0 ** (np.arange(0, HD, 2, dtype=np.float64) / HD))
    pos = np.arange(S, dtype=np.float64)
    sinu = pos[None, :] * inv_freq[:, None]          # [32, S]
    c = np.sin(sinu).astype(np.float32)              # torch code calls this 'cos'
    s = np.cos(sinu).astype(np.float32)              # and this 'sin'
    c2 = np.tile(c, (4, 1))                          # [128, S]
    s2 = np.concatenate([-s, s, -s, s], axis=0)      # [128, S]
    return np.ascontiguousarray(c2), np.ascontiguousarray(s2)


def make_in_maps(inp):
    """inp: dict of full numpy inputs -> list of 8 per-core input maps."""
    c2, s2 = _rope_tables()
    ones = np.ones((1, S), np.float32)
    pm = np.zeros((128, 128), np.float32)
    for h in range(2):
        for j in range(32):
            pm[h * 64 + 32 + j, h * 64 + j] = 1.0      # P[k, j]: out j <- in k
            pm[h * 64 + j, h * 64 + 32 + j] = 1.0
    XS = 256
    maps = []
    for c in range(NCORES):
        b, hg = c // 2, c % 2
        fsl = slice(hg * FC, (hg + 1) * FC)
        x = np.asarray(inp["hidden_states"][b], np.float32)
        xp = np.ascontiguousarray(
            x.reshape(S // XS, XS, NDC, 128).transpose(0, 3, 2, 1))
        wqp = np.ascontiguousarray(
            np.asarray(inp["Wq"], np.float32)[fsl].T.reshape(NDC, 128, FC)
            .transpose(1, 0, 2))
        wkp = np.ascontiguousarray(
            np.asarray(inp["Wk"], np.float32)[fsl].T.reshape(NDC, 128, FC)
            .transpose(1, 0, 2))
        wvp = np.ascontiguousarray(
            np.asarray(inp["Wv"], np.float32)[fsl].T.reshape(NDC, 128, FC)
            .transpose(1, 0, 2))
        wop = np.ascontiguousarray(
            np.asarray(inp["Wo"], np.float32)[:, fsl].T.reshape(NFC, 128, D)
            .transpose(1, 0, 2))
        maps.append({
            "xp": xp, "wq": wqp, "wk": wkp, "wv": wvp, "wo": wop,
            "bq": np.ascontiguousarray(np.asarray(inp["bq"], np.float32)[fsl]),
            "bk": np.ascontiguousarray(np.asarray(inp["bk"], np.float32)[fsl]),
            "bv": np.ascontiguousarray(
                np.asarray(inp["bv"], np.float32)[fsl][None, :]),
            "c2": c2, "s2": s2, "onesin": ones, "perm": pm,
        })
    return maps


_NC_CACHE = {}


def kernel(hidden_states, Wq, bq, Wk, bk, Wv, bv, Wo, bo):
    if "nc" not in _NC_CACHE:
        _NC_CACHE["nc"] = build_kernel()
    nc = _NC_CACHE["nc"]
    in_maps = make_in_maps({
        "hidden_states": hidden_states, "Wq": Wq, "bq": bq, "Wk": Wk, "bk": bk,
        "Wv": Wv, "bv": bv, "Wo": Wo,
    })
    res = run_bass_kernel_spmd(nc, in_maps, list(range(NCORES)))
    bo = np.asarray(bo, np.float32)
    out = np.empty((B, S, D), np.float32)
    for b in range(B):
        acc = res.results[2 * b]["yT"] + res.results[2 * b + 1]["yT"]
        out[b] = acc.T + bo[None, :]
    return out



# revision 2
# speedup vs baseline: 1.0082x; 1.0082x over previous
"""Multi-head attention (B=4, S=2048, D=1024, H=16, hd=64) with RoPE on 8 trn2 cores.

v2: bf16 matmul operands everywhere, QN=512 q-chunks, exp split between the
scalar engine (native Exp) and the vector engine (Schraudolph bit-trick into
bf16), output-projection copies on the scalar engine, bf16 output partials.

Sharding: core c handles batch b=c//2, head-group hg=c%2 (8 heads, 512
features). Each core computes y_partial.T = Wo[:, fslice] @ ctx.T for its
heads; the host sums the two partials per batch and adds bo.

Device layout (contraction dim on partitions):
  xp   [4, 128, 8, 512]  hidden.T slabs (host-pretransposed, bf16)
  wq/wk/wv [128, 8, 512] W[fslice,:].T chunks (bf16)
  Q.T/K.T [512, 2048]    per-core projections (bf16), RoPE applied
  V'   [S, 8, 65]        natural-layout V per head + ones column (bf16)
  scores.T [k, q] = K.T^T @ Q.T chunks -> exp(scale*s - 8) in bf16
  PV: ctx.T[hd+1, q] = V'^T @ expS.T  (row 64 = softmax denominator)
  out: y.T[e, q] = Wo.T^T @ (ctx.T * 1/den), partials in bf16
"""

import contextlib

import numpy as np
import ml_dtypes

import concourse.bass as bass
import concourse.mybir as mybir
import concourse.tile as tile
from concourse import bacc
from concourse.bass_utils import run_bass_kernel_spmd

F32 = mybir.dt.float32
BF16 = mybir.dt.bfloat16
I16 = mybir.dt.int16
AF = mybir.ActivationFunctionType
ADD = mybir.AluOpType.add
MULT = mybir.AluOpType.mult

B, S, D, H = 4, 2048, 1024, 16
HD = D // H            # 64
NCORES = 8
FC = D // 2            # 512 features (8 heads) per core (2 cores per batch)
NH = FC // HD          # 8 heads per core
QN = 512               # q-chunk width for attention/out-proj matmuls
XS = 512               # x slab width for projections
NQC = S // QN          # 4
NKC = S // 128         # 16 k-chunks
NDC = D // 128         # 8 d-chunks
NFC = FC // 128        # 4 f-chunks (head pairs)
EXP_BIAS = -8.0        # constant shift inside exp (cancels in softmax)
SCALE = 1.0 / np.sqrt(HD)
A16 = 128.0 / np.log(2.0)   # Schraudolph slope for bf16 bit pattern
BSH = 16250.5               # 127*128 - c_opt
ACT_EXP = (0, 2, 4, 5, 7, 9, 11, 13)  # g indices handled by ACT (rest DVE)


def build_kernel(dump=False, repeat=1, parts="all"):
    nc = bacc.Bacc("TRN2", debug=False)

    xp = nc.dram_tensor("xp", [S // XS, 128, NDC, XS], BF16, kind="ExternalInput")
    wq = nc.dram_tensor("wq", [128, NDC, FC], BF16, kind="ExternalInput")
    wk = nc.dram_tensor("wk", [128, NDC, FC], BF16, kind="ExternalInput")
    wv = nc.dram_tensor("wv", [128, NDC, FC], BF16, kind="ExternalInput")
    wo = nc.dram_tensor("wo", [128, NFC, D], BF16, kind="ExternalInput")
    bq = nc.dram_tensor("bq", [FC], F32, kind="ExternalInput")
    bk = nc.dram_tensor("bk", [FC], F32, kind="ExternalInput")
    bv = nc.dram_tensor("bv", [1, FC], BF16, kind="ExternalInput")
    c2 = nc.dram_tensor("c2", [128, S], BF16, kind="ExternalInput")
    s2 = nc.dram_tensor("s2", [128, S], BF16, kind="ExternalInput")
    onesin = nc.dram_tensor("onesin", [1, S], BF16, kind="ExternalInput")
    perm = nc.dram_tensor("perm", [128, 128], BF16, kind="ExternalInput")
    yT = nc.dram_tensor("yT", [D, S], BF16, kind="ExternalOutput")
    if dump:
        qT_d = nc.dram_tensor("qT_d", [FC, S], BF16, kind="ExternalOutput")
        kT_d = nc.dram_tensor("kT_d", [FC, S], BF16, kind="ExternalOutput")
        vt_d = nc.dram_tensor("vt_d", [S, NH, HD + 1], BF16, kind="ExternalOutput")

    with tile.TileContext(nc) as tc:
      for _rep in range(repeat):
       with contextlib.ExitStack() as ctx:
        ll = ctx.enter_context(tc.tile_pool(name="ll", bufs=1))

        qT = [ll.tile([128, S], BF16, name=f"qT{i}") for i in range(NFC)]
        kT = [ll.tile([128, S], BF16, name=f"kT{i}") for i in range(NFC)]
        ebias = ll.tile([128, 1], F32, name="ebias")
        ones_sb = ll.tile([1, S], BF16, name="ones_sb")
        ones_col = ll.tile([128, NH], BF16, name="ones_col")
        perm_sb = ll.tile([128, 128], BF16, name="perm_sb")
        bqs = ll.tile([128, NFC], F32, name="bqs")
        bks = ll.tile([128, NFC], F32, name="bks")
        wo_sb = ll.tile([128, NFC, D], BF16, name="wo_sb")
        nc.sync.dma_start(out=wo_sb, in_=wo[:])

        nc.vector.memset(ebias, EXP_BIAS)
        nc.sync.dma_start(out=ones_sb, in_=onesin[:])
        ones_dram = onesin[:]
        nc.sync.dma_start(
            out=ones_col,
            in_=bass.AP(tensor=ones_dram.tensor, offset=ones_dram.offset,
                        ap=[[0, 128], [1, NH]]))
        nc.sync.dma_start(out=perm_sb, in_=perm[:])
        nc.sync.dma_start(out=bqs, in_=bq[:].rearrange("(c p) -> p c", p=128))
        nc.sync.dma_start(out=bks, in_=bk[:].rearrange("(c p) -> p c", p=128))

        # ---------------- Phase A: Q/K/V projections + RoPE (one xp pass) ----------------
        ll2 = ctx.enter_context(tc.tile_pool(name="ll2", bufs=1))
        vt = [ll2.tile([128, NH, HD + 1], BF16, name=f"vt{k}") for k in range(NKC)]
        xp_r = xp[:]
        if parts == "3":
            # skip phase A: fill qT/kT/vt from the weight dram tensors so
            # phase 3 runs standalone with realistic data deps
            for fc in range(NFC):
                nc.sync.dma_start(
                    out=qT[fc],
                    in_=wq[:, 0:4, :].rearrange("p a f -> p (a f)"))
                nc.sync.dma_start(
                    out=kT[fc],
                    in_=wk[:, 4:8, :].rearrange("p a f -> p (a f)"))
            for kg in range(NKC):
                nc.sync.dma_start(
                    out=vt[kg].rearrange("p h e -> p (h e)")[:, 0:FC],
                    in_=wv[:, kg % NDC, :])
                nc.vector.tensor_copy(
                    vt[kg][:, :, HD:HD + 1],
                    ones_col.rearrange("p (h o) -> p h o", o=1))
        with tc.tile_pool(name="pA", bufs=1) as pA, \
             tc.tile_pool(name="ppA", bufs=1, space="PSUM") as ppA:
          if parts != "3":
            c2_sb = pA.tile([128, S], BF16, name="c2_sb")
            s2_sb = pA.tile([128, S], BF16, name="s2_sb")
            nc.sync.dma_start(out=c2_sb, in_=c2[:])
            nc.sync.dma_start(out=s2_sb, in_=s2[:])
            with tc.tile_pool(name="wqk", bufs=1) as wqk:
                wq_sb = wqk.tile([128, NDC, FC], BF16, name="wq_sb")
                wk_sb = wqk.tile([128, NDC, FC], BF16, name="wk_sb")
                wv_sb = wqk.tile([128, NDC, FC], BF16, name="wv_sb")
                bv_sb = wqk.tile([1, FC], BF16, name="bv_sb")
                nc.sync.dma_start(out=bv_sb, in_=bv[:])
                nc.sync.dma_start(out=wq_sb, in_=wq[:])
                nc.sync.dma_start(out=wk_sb, in_=wk[:])
                nc.sync.dma_start(out=wv_sb, in_=wv[:])

                nmove = 0
                for scg in range(S // XS):
                    xh = pA.tile([128, NDC, XS], BF16, name="xh", tag="xh", bufs=2)
                    nc.sync.dma_start(out=xh, in_=xp_r[scg])
                    sg = scg * XS
                    for fc in range(NFC):
                        for w_sb, bias_t, out_t in (
                            (wq_sb, bqs, qT),
                            (wk_sb, bks, kT),
                        ):
                            ps = ppA.tile([128, XS], F32, name="ps", tag="proj",
                                          bufs=3)
                            for d in range(NDC):
                                nc.tensor.matmul(
                                    ps,
                                    w_sb[:, d, fc * 128:(fc + 1) * 128],
                                    xh[:, d, :],
                                    start=(d == 0),
                                    stop=(d == NDC - 1),
                                )
                            t = out_t[fc]
                            praw = pA.tile(
                                [128, XS], BF16, name="praw", tag="praw", bufs=3,
                            )
                            # PSUM->SBUF move + bias, alternating ACT/DVE
                            if nmove % 8 < 5:
                                nc.scalar.activation(
                                    praw, ps, AF.Identity,
                                    bias=bias_t[:, fc:fc + 1], scale=1.0,
                                )
                            else:
                                nc.vector.tensor_scalar(
                                    praw, ps, bias_t[:, fc:fc + 1], None, op0=ADD,
                                )
                            nmove += 1
                            # RoPE: t_slice = praw*C2 + (P.T @ praw)*S2
                            swps = ppA.tile(
                                [128, XS], F32, name="swps", tag="swp", bufs=2,
                            )
                            nc.tensor.matmul(
                                swps, perm_sb, praw, start=True, stop=True,
                            )
                            prod = pA.tile(
                                [128, XS], BF16, name="prod", tag="prod", bufs=3,
                            )
                            nc.vector.tensor_mul(
                                prod, swps, s2_sb[:, sg:sg + XS])
                            nc.gpsimd.tensor_mul(
                                t[:, sg:sg + XS], praw, c2_sb[:, sg:sg + XS])
                            nc.gpsimd.tensor_add(
                                t[:, sg:sg + XS], t[:, sg:sg + XS], prod)
                    for ss in range(XS // 128):
                        kg = scg * (XS // 128) + ss
                        psv = ppA.tile([128, FC], F32, name="psv", tag="projv",
                                       bufs=2)
                        for d in range(NDC):
                            nc.tensor.matmul(
                                psv,
                                xh[:, d, ss * 128:(ss + 1) * 128],
                                wv_sb[:, d, :],
                                start=(d == 0),
                                stop=False,
                            )
                        nc.tensor.matmul(
                            psv, ones_sb[0:1, 0:128], bv_sb, start=False, stop=True,
                        )
                        nc.scalar.activation(
                            vt[kg][:, :, 0:HD],
                            psv.rearrange("p (h e) -> p h e", e=HD),
                            AF.Copy,
                        )
                        nc.gpsimd.tensor_copy(
                            vt[kg][:, :, HD:HD + 1],
                            ones_col.rearrange("p (h o) -> p h o", o=1),
                        )
            if dump:
                qd_r = qT_d[:].rearrange("(c p) s -> c p s", p=128)
                kd_r = kT_d[:].rearrange("(c p) s -> c p s", p=128)
                for fc in range(NFC):
                    nc.sync.dma_start(out=qd_r[fc], in_=qT[fc])
                    nc.sync.dma_start(out=kd_r[fc], in_=kT[fc])
                for kg in range(NKC):
                    nc.sync.dma_start(
                        out=vt_d[:].rearrange("(c p) h e -> c p h e", p=128)[kg],
                        in_=vt[kg],
                    )

        if parts == "A":
            # consume qT/kT/vt via DMA to the output so phase A times alone
            y_r = yT[:].rearrange("(c p) s -> c p s", p=128)
            for fc in range(NFC):
                nc.sync.dma_start(out=y_r[fc], in_=qT[fc])
                nc.sync.dma_start(out=y_r[NFC + fc], in_=kT[fc])
            for kg in range(NKC):
                nc.sync.dma_start(
                    out=y_r[0, :, kg * 128:kg * 128 + 32],
                    in_=vt[kg].rearrange("p h e -> p (h e)")[:, 0:32])
            continue

        # ---------------- Phase 3: attention + output projection ----------------
        NG = NKC // 2          # 8 double-chunk groups per (pair, h)
        with tc.tile_pool(name="p3", bufs=1) as p3, \
             tc.tile_pool(name="ps_sc", bufs=2, space="PSUM") as ps_sc, \
             tc.tile_pool(name="ps_pv", bufs=2, space="PSUM") as ps_pv, \
             tc.tile_pool(name="ps_o", bufs=2, space="PSUM") as ps_o:
            for qc in range(NQC):
                q_sl = slice(qc * QN, (qc + 1) * QN)
                ctxT = []
                for pair in range(NFC):
                    exp_sl = [[None] * NG for _ in range(2)]
                    for g in range(NG):
                        sc_ps = [
                            ps_sc.tile([128, 2, QN], F32, name="sc_ps", tag="sc")
                            for h in range(2)
                        ]
                        for j in range(2):
                            kc = g * 2 + j
                            k_sl = slice(kc * 128, (kc + 1) * 128)
                            for h in range(2):
                                nc.tensor.matmul(
                                    sc_ps[h][:, j, :],
                                    kT[pair][h * 64:(h + 1) * 64, k_sl],
                                    qT[pair][h * 64:(h + 1) * 64, q_sl],
                                    start=True, stop=True,
                                    tile_position=(h * 64, 0),
                                )
                        for h in range(2):
                            es = p3.tile(
                                [128, 2, QN], BF16, name="es", tag=f"exp{h}{g}",
                                bufs=2,
                            )
                            # ACT-heavy split: DVE ops pay a pipeline DRAIN
                            # (~2x their duration), so DVE takes only ~20 of
                            # the 64 exp chunks per q-block.
                            on_act = not (h == 1 and (pair < 2
                                                      or (pair == 2 and g < 4)))
                            if on_act:
                                nc.scalar.activation(
                                    es, sc_ps[h], AF.Exp, bias=ebias, scale=SCALE,
                                )
                            else:
                                nc.vector.tensor_scalar(
                                    es.bitcast(I16), sc_ps[h],
                                    A16 * SCALE, A16 * EXP_BIAS + BSH,
                                    op0=MULT, op1=ADD,
                                )
                            exp_sl[h][g] = es
                    for h in range(2):
                        hh = pair * 2 + h
                        pv_ps = ps_pv.tile([128, QN], F32, name="pv_ps", tag="pv")
                        for kc in range(NKC):
                            nc.tensor.matmul(
                                pv_ps[0:HD + 1, :],
                                vt[kc][:, hh, :],
                                exp_sl[h][kc // 2][:, kc % 2, :],
                                start=(kc == 0),
                                stop=(kc == NKC - 1),
                            )
                        denr = p3.tile([1, QN], F32, name="denr", tag="denr")
                        nc.vector.reciprocal(denr, pv_ps[HD:HD + 1, :])
                        denb = p3.tile([64, QN], F32, name="denb", tag="denb")
                        nc.gpsimd.partition_broadcast(denb, denr)
                        if h == 0:
                            ct = p3.tile([128, QN], BF16, name="ct",
                                         tag=f"ctx{pair}")
                            ctxT.append(ct)
                        nc.vector.tensor_tensor(
                            ctxT[pair][h * 64:(h + 1) * 64, :],
                            pv_ps[0:HD, :], denb, op=MULT,
                        )
                for ec in range(NDC):
                    ops = ps_o.tile([128, QN], F32, name="ops", tag="out")
                    for fc in range(NFC):
                        nc.tensor.matmul(
                            ops,
                            wo_sb[:, fc, ec * 128:(ec + 1) * 128],
                            ctxT[fc],
                            start=(fc == 0),
                            stop=(fc == NFC - 1),
                        )
                    ysb = p3.tile([128, QN], BF16, name="ysb", tag="y", bufs=3)
                    nc.scalar.activation(ysb, ops, AF.Copy)
                    nc.sync.dma_start(
                        out=yT[:].rearrange("(c p) s -> c p s", p=128)[ec, :, q_sl],
                        in_=ysb,
                    )

    nc.finalize()
    return nc


def _rope_tables():
    inv_freq = 1.0 / (10000.0 ** (np.arange(0, HD, 2, dtype=np.float64) / HD))
    pos = np.arange(S, dtype=np.float64)
    sinu = pos[None, :] * inv_freq[:, None]          # [32, S]
    c = np.sin(sinu).astype(np.float32)              # torch code calls this 'cos'
    s = np.cos(sinu).astype(np.float32)              # and this 'sin'
    c2 = np.tile(c, (4, 1))                          # [128, S]
    s2 = np.concatenate([-s, s, -s, s], axis=0)      # [128, S]
    return (np.ascontiguousarray(c2).astype(ml_dtypes.bfloat16),
            np.ascontiguousarray(s2).astype(ml_dtypes.bfloat16))


def make_in_maps(inp):
    """inp: dict of full numpy inputs -> list of 8 per-core input maps."""
    c2, s2 = _rope_tables()
    ones = np.ones((1, S), ml_dtypes.bfloat16)
    pm = np.zeros((128, 128), ml_dtypes.bfloat16)
    for h in range(2):
        for j in range(32):
            pm[h * 64 + 32 + j, h * 64 + j] = 1.0      # P[k, j]: out j <- in k
            pm[h * 64 + j, h * 64 + 32 + j] = 1.0
    bf = ml_dtypes.bfloat16
    maps = []
    for c in range(NCORES):
        b, hg = c // 2, c % 2
        fsl = slice(hg * FC, (hg + 1) * FC)
        x = np.asarray(inp["hidden_states"][b], np.float32)
        xp = np.ascontiguousarray(
            x.reshape(S // XS, XS, NDC, 128).transpose(0, 3, 2, 1)).astype(bf)
        wqp = np.ascontiguousarray(
            np.asarray(inp["Wq"], np.float32)[fsl].T.reshape(NDC, 128, FC)
            .transpose(1, 0, 2)).astype(bf)
        wkp = np.ascontiguousarray(
            np.asarray(inp["Wk"], np.float32)[fsl].T.reshape(NDC, 128, FC)
            .transpose(1, 0, 2)).astype(bf)
        wvp = np.ascontiguousarray(
            np.asarray(inp["Wv"], np.float32)[fsl].T.reshape(NDC, 128, FC)
            .transpose(1, 0, 2)).astype(bf)
        wop = np.ascontiguousarray(
            np.asarray(inp["Wo"], np.float32)[:, fsl].T.reshape(NFC, 128, D)
            .transpose(1, 0, 2)).astype(bf)
        maps.append({
            "xp": xp, "wq": wqp, "wk": wkp, "wv": wvp, "wo": wop,
            "bq": np.ascontiguousarray(np.asarray(inp["bq"], np.float32)[fsl]),
            "bk": np.ascontiguousarray(np.asarray(inp["bk"], np.float32)[fsl]),
            "bv": np.ascontiguousarray(
                np.asarray(inp["bv"], np.float32)[fsl][None, :]).astype(bf),
            "c2": c2, "s2": s2, "onesin": ones, "perm": pm,
        })
    return maps


_NC_CACHE = {}


def kernel(hidden_states, Wq, bq, Wk, bk, Wv, bv, Wo, bo):
    if "nc" not in _NC_CACHE:
        _NC_CACHE["nc"] = build_kernel()
    nc = _NC_CACHE["nc"]
    in_maps = make_in_maps({
        "hidden_states": hidden_states, "Wq": Wq, "bq": bq, "Wk": Wk, "bk": bk,
        "Wv": Wv, "bv": bv, "Wo": Wo,
    })
    res = run_bass_kernel_spmd(nc, in_maps, list(range(NCORES)))
    bo = np.asarray(bo, np.float32)
    out = np.empty((B, S, D), np.float32)
    for b in range(B):
        acc = (res.results[2 * b]["yT"].astype(np.float32)
               + res.results[2 * b + 1]["yT"].astype(np.float32))
        out[b] = acc.T + bo[None, :]
    return out
